# revision 1
# baseline (speedup 1.0000x reference)
"""Trainium2 Bass kernel for nn_GatFeatDecoder (GAT-style decoder).

Reference computation per batch b (B=16, W=64, K=256, E=128, O=64):
    v = x[b].T                               (K, W)
    l = v @ W1.T ; r = v @ W2.T              (K, E) each
    e[i,j]  = sum_e a_e * LeakyReLU(l[i,e] + r[j,e] + lin_b[e]) + bias_kk[i,j]
    attn    = softmax_j(e)
    h       = sigmoid(attn @ v)              (K, W)
    out[b]  = h.T @ fc_w.T + fc_b            (W, O)

Kernel strategy (data-parallel, 2 batches per core on 8 cores, no
collectives):
  * Fold (1-alpha)*|a| into W1/W2/lin_b rows => z~ = (1-alpha)|a| z, and
    sum_e a_e*LeakyReLU(z) = sum_e sgn_e relu(z~) + alpha' * sum_e sgn_e z~
    with alpha' = alpha/(1-alpha).  The per-i part of the linear term is
    constant across j and cancels in softmax; the per-j part is
    srb_j = alpha' * (sgn . rtb[:,j]), computed on device with one matmul.
  * relu tiles T^J[e, i] = relu(lt[e,i] + rtb[e,J]) are produced per key
    node J with one DVE tensor_scalar (add + max0, bf16 4x mode) or one
    ACT activation (Relu with per-partition bias) - work split between
    the two engines.
  * The e-contraction sum_e sgn_e T^J[e,i] is one M=1 matmul per J with
    the +-1 vector as stationary weights, 4-way column-tiled so four J
    streams run concurrently on the PE array.  Rows land in PSUM at
    partitions {0,32,64,96} x 2 free halves; one DMA per 8 J's scatters
    them into score^T tiles [j, i] in SBUF (identity j-order).
  * softmax without per-row max (logits bounded): P^T = exp(score^T +
    srb_j + bias_kk^T).  attn@v and the row-sum denominator come from one
    matmul per (i-half, j-half) with lhsT = P^T and rhs = [v | 2.0]; then
    h = sigmoid(num/den) = 0.5*(tanh(num * (0.5/den)) + 1), realized as
    ACT Tanh with per-partition scale = reciprocal(2*sum exp), with the
    0.5/0.5 affine folded into the fc weights/bias on the host.
  * fc: out^T[o,w] = sum_k (0.5 fc_w)^T t + (fc_b + 0.5 sum_k fc_w).
"""

import numpy as np

import concourse.bass as bass
import concourse.bacc as bacc
import concourse.tile as tile
from concourse import mybir
from concourse.bass_utils import run_bass_kernel_spmd

ALPHA = 0.2
B, Wn, K, E, O = 16, 64, 256, 128, 64
N_CORES = 8
BPC = B // N_CORES  # batches per core

FP32 = mybir.dt.float32
BF16 = mybir.dt.bfloat16
AF = mybir.ActivationFunctionType
ALU = mybir.AluOpType

# fraction of relu-tile generation sent to DVE (rest to ACT)
DVE_SHARE_MOD = 4  # J % DVE_SHARE_MOD == DVE_SHARE_MOD-1 -> ACT


def _build_program():
    nc = bacc.Bacc("TRN2", target_bir_lowering=False, debug=False,
                   num_devices=N_CORES)

    # ---- per-core DRAM I/O ----
    d_x = nc.dram_tensor("xin", [BPC, Wn, K], FP32, kind="ExternalInput")
    d_xto2 = nc.dram_tensor("xto2", [BPC, K, Wn + 1], FP32, kind="ExternalInput")
    d_w1at = nc.dram_tensor("w1at", [Wn, E], FP32, kind="ExternalInput")
    d_w2bt = nc.dram_tensor("w2bt", [Wn + 1, E], FP32, kind="ExternalInput")
    d_sgnw = nc.dram_tensor("sgnw", [E, 1024], FP32, kind="ExternalInput")
    d_asgn = nc.dram_tensor("asgnv", [E, 1], FP32, kind="ExternalInput")
    d_bkkt = nc.dram_tensor("bkkt", [K, K], FP32, kind="ExternalInput")
    d_fcw2t = nc.dram_tensor("fcw2t", [K, O], FP32, kind="ExternalInput")
    d_fcb2 = nc.dram_tensor("fcb2", [O, 1], FP32, kind="ExternalInput")
    d_out = nc.dram_tensor("outp", [BPC, O, Wn], FP32, kind="ExternalOutput")

    with tile.TileContext(nc) as tc:
        with (
            tc.tile_pool(name="consts", bufs=1) as consts,
            tc.tile_pool(name="setup", bufs=2) as setup,
            tc.tile_pool(name="trelu", bufs=16) as trelu,
            tc.tile_pool(name="etiles", bufs=4) as etiles,
            tc.tile_pool(name="small", bufs=8) as small,
            tc.tile_pool(name="psA", bufs=1, space="PSUM") as psA,
            tc.tile_pool(name="psS", bufs=2, space="PSUM") as psS,
            tc.tile_pool(name="psV", bufs=1, space="PSUM") as psV,
            tc.tile_pool(name="psF", bufs=1, space="PSUM") as psF,
            tc.tile_pool(name="psR", bufs=1, space="PSUM") as psR,
        ):
            # ---------- load constants ----------
            w1at = consts.tile([Wn, E], FP32, tag="w1at")
            nc.gpsimd.dma_start(out=w1at[:], in_=d_w1at.ap())
            w2bt = consts.tile([Wn + 1, E], FP32, tag="w2bt")
            nc.gpsimd.dma_start(out=w2bt[:], in_=d_w2bt.ap())
            sgnw_f = consts.tile([E, 1024], FP32, tag="sgnwf")
            nc.gpsimd.dma_start(out=sgnw_f[:], in_=d_sgnw.ap())
            asgn_f = consts.tile([E, 1], FP32, tag="asgnf")
            nc.gpsimd.dma_start(out=asgn_f[:], in_=d_asgn.ap())
            bkkt = [consts.tile([K // 2, K], FP32, tag=f"bkkt{t}", name=f"bkkt{t}") for t in range(2)]
            for t in range(2):
                nc.gpsimd.dma_start(out=bkkt[t][:], in_=d_bkkt.ap()[128 * t:128 * t + 128, :])
            fcw2t = [consts.tile([K // 2, O], FP32, tag=f"fcw2t{h}",
                                 name=f"fcw2t{h}") for h in range(2)]
            for h in range(2):
                nc.gpsimd.dma_start(out=fcw2t[h][:],
                                  in_=d_fcw2t.ap()[128 * h:128 * h + 128, :])
            fcb2 = consts.tile([O, 1], FP32, tag="fcb2")
            nc.gpsimd.dma_start(out=fcb2[:], in_=d_fcb2.ap())

            # bf16 casts (also funnels PE inputs through DVE so PE
            # instructions wait on a single semaphore)
            w1at_b = consts.tile([Wn, E], BF16, tag="w1atb")
            nc.vector.tensor_copy(w1at_b[:], w1at[:])
            w2bt_b = consts.tile([Wn + 1, E], BF16, tag="w2btb")
            nc.vector.tensor_copy(w2bt_b[:], w2bt[:])
            sgnw_b = consts.tile([E, 1024], BF16, tag="sgnwb")
            nc.vector.tensor_copy(sgnw_b[:], sgnw_f[:])
            asgn_b = consts.tile([E, 1], BF16, tag="asgnb")
            nc.vector.tensor_copy(asgn_b[:], asgn_f[:])
            fcw2t_b = [consts.tile([K // 2, O], BF16, tag=f"fcw2tb{h}",
                                   name=f"fcw2tb{h}") for h in range(2)]
            for h in range(2):
                nc.vector.tensor_copy(fcw2t_b[h][:], fcw2t[h][:])

            for b in range(BPC):
                # ---------- setup: lt / rtb projections ----------
                xb = setup.tile([Wn + 1, K], FP32, tag="xb")
                nc.gpsimd.dma_start(out=xb[0:Wn, :], in_=d_x.ap()[b])
                nc.vector.memset(xb[Wn:Wn + 1, :], 1.0)
                xb_b = setup.tile([Wn + 1, K], BF16, tag="xb_b")
                nc.vector.tensor_copy(xb_b[:], xb[:])

                ps_lt = psA.tile([E, K], FP32, tag="ps_lt")
                nc.tensor.matmul(ps_lt[:], w1at_b[:], xb_b[0:Wn, :],
                                 start=True, stop=True)
                ps_rt = psA.tile([E, K], FP32, tag="ps_rt")
                nc.tensor.matmul(ps_rt[:], w2bt_b[:], xb_b[:],
                                 start=True, stop=True)

                lt_b = setup.tile([E, K], BF16, tag="lt_b")
                nc.scalar.copy(lt_b[:], ps_lt[:])
                rtb_f = setup.tile([E, K], FP32, tag="rtb_f")
                nc.scalar.copy(rtb_f[:], ps_rt[:])
                rtb_b = setup.tile([E, K], BF16, tag="rtb_b")
                nc.vector.tensor_copy(rtb_b[:], ps_rt[:])

                # srb[j] = alpha' * sum_e sgn_e rtb[e, j]  (column, per j-half)
                ps_srb = psR.tile([K // 2, 2], FP32, tag="ps_srb")
                for t in range(2):
                    nc.tensor.matmul(ps_srb[:, t:t + 1],
                                     rtb_b[:, 128 * t:128 * t + 128],
                                     asgn_b[:], start=True, stop=True)
                srb = small.tile([K // 2, 2], FP32, tag="srb")
                nc.vector.tensor_copy(srb[:], ps_srb[:])

                # xto2 (rhs for attn@v), cast to bf16 per j-half
                xto = [small.tile([K // 2, Wn + 1], BF16, tag=f"xto{h}", name=f"xto{h}")
                       for h in range(2)]
                for h in range(2):
                    xf = small.tile([K // 2, Wn + 1], FP32, tag=f"xtof{h}")
                    nc.gpsimd.dma_start(
                        out=xf[:], in_=d_xto2.ap()[b, 128 * h:128 * h + 128, :])
                    nc.vector.tensor_copy(xto[h][:], xf[:])

                # ---------- relu-gen + scatter-contraction ----------
                # scores^T tile [j-half=128, i=256] accumulates directly in
                # PSUM: the matmul for J uses a [128, 32] weight tile whose
                # only nonzero column (at index m = J%32) is the sign
                # vector, so the score row lands on partition 32g + m.
                pT = [etiles.tile([K // 2, K], BF16, tag=f"pT{t}", name=f"pT{t}") for t in range(2)]
                for jh in range(2):
                    ps_sc = psS.tile([128, K], FP32, tag="ps_sc")
                    for m in range(32):
                        for g in range(4):
                            J = 128 * jh + 32 * g + m
                            tj = trelu.tile([E, K], BF16, tag="tj")
                            if m % 5 == 2:
                                nc.scalar.activation(
                                    tj[:], lt_b[:], AF.Relu,
                                    bias=rtb_f[:, J:J + 1], scale=1.0)
                            else:
                                nc.vector.tensor_scalar(
                                    out=tj[:], in0=lt_b[:],
                                    scalar1=rtb_f[:, J:J + 1], scalar2=0.0,
                                    op0=ALU.add, op1=ALU.max)
                            nc.tensor.matmul(
                                ps_sc[32 * g:32 * g + 32, :],
                                sgnw_b[:, 32 * m:32 * m + 32], tj[:],
                                start=(m == 0), stop=(m == 31),
                                tile_position=(0, 32 * g),
                                skip_group_check=True)
                    # logits -> P^T = exp(S + srb_j + bias_kk^T)
                    et = etiles.tile([K // 2, K], FP32, tag="et")
                    nc.vector.scalar_tensor_tensor(
                        out=et[:], in0=ps_sc[:], scalar=srb[:, jh:jh + 1],
                        in1=bkkt[jh][:], op0=ALU.add, op1=ALU.add)
                    nc.scalar.activation(pT[jh][:], et[:], AF.Exp)

                # ---------- attn @ [v | 2] ----------
                t_ih = []
                for ih in range(2):
                    ps_num = psV.tile([K // 2, Wn + 1], FP32, tag="ps_num")
                    for jh in range(2):
                        nc.tensor.matmul(
                            ps_num[:], pT[jh][:, 128 * ih:128 * ih + 128],
                            xto[jh][:], start=(jh == 0), stop=(jh == 1))
                    rcol = small.tile([K // 2, 1], FP32, tag=f"rcol{ih}")
                    nc.vector.reciprocal(rcol[:], ps_num[:, Wn:Wn + 1])
                    tt = small.tile([K // 2, Wn], BF16, tag=f"tt{ih}")
                    nc.scalar.activation(tt[:], ps_num[:, 0:Wn], AF.Tanh,
                                         scale=rcol[:])
                    t_ih.append(tt)

                # ---------- fc: out^T = (0.5 fc_w)^T t + fcb2 ----------
                ps_o = psF.tile([O, Wn], FP32, tag="ps_o")
                for ih in range(2):
                    nc.tensor.matmul(ps_o[:], fcw2t_b[ih][:],
                                     t_ih[ih][:], start=(ih == 0), stop=(ih == 1))
                ot = small.tile([O, Wn], FP32, tag="ot")
                nc.scalar.activation(ot[:], ps_o[:], AF.Identity, bias=fcb2[:])
                nc.gpsimd.dma_start(out=d_out.ap()[b], in_=ot[:])

    nc.compile()
    return nc


_NC_CACHE = {}


def _get_program():
    if "nc" not in _NC_CACHE:
        _NC_CACHE["nc"] = _build_program()
    return _NC_CACHE["nc"]


def _host_prep(x, lin_w, lin_b, a, bias_kk, fc_w, fc_b):
    f32 = np.float32
    x = np.ascontiguousarray(x, f32)
    aa = (np.abs(a) * (1.0 - ALPHA)).astype(f32)
    sgn = np.sign(a).astype(f32)
    w1at = np.ascontiguousarray((lin_w[:, :Wn] * aa[:, None]).T, f32)
    w2t = (lin_w[:, Wn:] * aa[:, None]).T
    bt = (lin_b * aa)[None, :]
    w2bt = np.ascontiguousarray(np.concatenate([w2t, bt], 0), f32)
    xto2 = np.concatenate(
        [np.transpose(x, (0, 2, 1)),
         np.full((B, K, 1), 2.0, f32)], axis=2)
    xto2 = np.ascontiguousarray(xto2, f32)
    bkkt = np.ascontiguousarray(bias_kk.T, f32)
    fcw2t = np.ascontiguousarray((0.5 * fc_w).T, f32)
    fcb2 = np.ascontiguousarray(
        (fc_b + 0.5 * fc_w.sum(1)).reshape(O, 1), f32)
    sgnw = np.zeros((E, 1024), f32)
    for m in range(32):
        sgnw[:, 32 * m + m] = sgn
    shared = dict(w1at=w1at, w2bt=w2bt, sgnw=sgnw,
                  asgnv=np.ascontiguousarray((0.25 * sgn).reshape(E, 1)),
                  bkkt=bkkt, fcw2t=fcw2t, fcb2=fcb2)
    in_maps = []
    for c in range(N_CORES):
        m = dict(shared)
        m["xin"] = np.ascontiguousarray(x[BPC * c:BPC * (c + 1)])
        m["xto2"] = np.ascontiguousarray(xto2[BPC * c:BPC * (c + 1)])
        in_maps.append(m)
    return in_maps


def kernel(x, lin_w, lin_b, a, bias_kk, fc_w, fc_b, _trace=False):
    nc = _get_program()
    in_maps = _host_prep(np.asarray(x), np.asarray(lin_w), np.asarray(lin_b),
                         np.asarray(a), np.asarray(bias_kk),
                         np.asarray(fc_w), np.asarray(fc_b))
    res = run_bass_kernel_spmd(nc, in_maps, list(range(N_CORES)),
                               trace=_trace)
    out = np.empty((B, Wn, O), np.float32)
    for c in range(N_CORES):
        o = res.results[c]["outp"]          # (BPC, O, Wn)
        for i in range(BPC):
            out[BPC * c + i] = o[i].T
    if _trace:
        return out, res
    return out



# revision 6
# speedup vs baseline: 1.2780x; 1.2780x over previous
"""Trainium2 Bass kernel for nn_GatFeatDecoder (GAT-style decoder).

Reference computation per batch b (B=16, W=64, K=256, E=128, O=64):
    v = x[b].T                               (K, W)
    l = v @ W1.T ; r = v @ W2.T              (K, E) each
    e[i,j]  = sum_e a_e * LeakyReLU(l[i,e] + r[j,e] + lin_b[e]) + bias_kk[i,j]
    attn    = softmax_j(e)
    h       = sigmoid(attn @ v)              (K, W)
    out[b]  = h.T @ fc_w.T + fc_b            (W, O)

Data-parallel: 2 batches per core on 8 cores, no collectives.

Math folding (per-core):
  * z~ = (1-a)|a_e| (l+r+b); sum_e a_e LeakyReLU = sum_e sgn_e relu(z~)
    + alpha' * sum_e sgn_e z~, alpha' = alpha/(1-alpha).  The per-i part
    of the linear term cancels in softmax; the per-j part srb_j =
    sum_w q_w xb[w,j] with q = alpha' * W2b @ sgn precomputed on host,
    and enters as the per-partition bias of the exp() activation.
  * bias_kk^T is accumulated into the score PSUM tile by one extra
    matmul with identity weights, so exp() reads PSUM directly.
  * relu tiles T^J[e,i] = relu(lt[e,i] + rtb[e,J]) are produced by a
    3-way DVE / ACT / Pool split (one instr per J).
  * score row J comes from a matmul whose [128,128] weight slice (from a
    sliding window of a sign-master matrix) has the sign vector at
    column j = J mod 128; 128 J-matmuls + the bias matmul form one PSUM
    accumulation chain per (batch, j-half).
  * softmax without row-max (logits bounded): P^T = exp(S^T + srb_j).
    attn@v and the denominator come from matmuls with rhs = [v | 2.0];
    h = sigmoid(num/den) = 0.5*(tanh(num * (0.5/den)) + 1) via ACT Tanh
    with per-partition scale = reciprocal(2*sum exp); the 0.5/0.5 affine
    is folded into the fc weights/bias on the host.
  * All constants + inputs arrive in two packed bf16 DMAs; outputs for
    both batches leave in one DMA.

Emission is software-pipelined across the two batches: batch-1 tile
generation is issued before batch-0's attn/fc epilogue so no engine
queue stalls at the batch boundary.
"""

import numpy as np
import ml_dtypes

import concourse.bass as bass
import concourse.bacc as bacc
import concourse.tile as tile
from concourse import mybir
from concourse.bass_utils import run_bass_kernel_spmd

ALPHA = 0.2
B, Wn, K, E, O = 16, 64, 256, 128, 64
N_CORES = 8
BPC = B // N_CORES  # batches per core

FP32 = mybir.dt.float32
BF16 = mybir.dt.bfloat16
AF = mybir.ActivationFunctionType
ALU = mybir.AluOpType

# ---- packed-constant column layout (bf16, 128 partitions) ----
# pack A (early: needed for batch-0 projections + first relu tiles)
A_W1 = 0                      # w1at   [64,128]  rows 0:64
A_W2 = A_W1 + E               # w2bt   [65,128]  rows 0:65
A_XB0 = A_W2 + E              # xb b0  [65,256]  rows 0:65 (row 64 = ones)
A_Q = A_XB0 + K               # q      [65,1]
A_SS = A_Q + 1                # sign master [128,256] (col 128 = sgn)
A_COLS = A_SS + 256

# pack B (late: not needed before ~10us into the kernel)
B_XB1 = 0                     # xb b1  [65,256]
B_XTO = B_XB1 + K             # xto2   4 x [128,65]  (b,h) = [v | 2.0]
B_BKT = B_XTO + 4 * (Wn + 1)  # bkkt^T 2 x [128,256]
B_ID = B_BKT + 2 * K          # identity [128,128]
B_FCW = B_ID + E              # fcw2t  2 x [128,64]
B_FCB = B_FCW + 2 * O         # fcb2   [64,1]
B_COLS = B_FCB + 1

# tile-gen engine split: per j-half index idx in 0..127
# idx%16 in {3,8,13} -> Pool, {5,10,15} -> ACT, else DVE
POOL_SET = {3, 8, 13}
ACT_SET = {5, 10, 15}


def _build_program():
    nc = bacc.Bacc("TRN2", target_bir_lowering=False, debug=False,
                   num_devices=N_CORES)

    d_packA = nc.dram_tensor("packA", [128, A_COLS], BF16, kind="ExternalInput")
    d_packB = nc.dram_tensor("packB", [128, B_COLS], BF16, kind="ExternalInput")
    d_out = nc.dram_tensor("outp", [O, BPC * Wn], FP32, kind="ExternalOutput")

    with tile.TileContext(nc) as tc:
        with (
            tc.tile_pool(name="consts", bufs=1) as consts,
            tc.tile_pool(name="setup", bufs=2) as setup,
            tc.tile_pool(name="trelu", bufs=24) as trelu,
            tc.tile_pool(name="etiles", bufs=4) as etiles,
            tc.tile_pool(name="small", bufs=8) as small,
            tc.tile_pool(name="psA", bufs=2, space="PSUM") as psA,
            tc.tile_pool(name="psS", bufs=2, space="PSUM") as psS,
            tc.tile_pool(name="psM", bufs=2, space="PSUM") as psM,
        ):
            cA = consts.tile([128, A_COLS], BF16, tag="cA")
            nc.gpsimd.dma_start(out=cA[:], in_=d_packA.ap())
            cB = consts.tile([128, B_COLS], BF16, tag="cB")
            nc.gpsimd.dma_start(out=cB[:], in_=d_packB.ap())

            w1at_v = cA[0:Wn, A_W1:A_W1 + E]
            w2bt_v = cA[0:Wn + 1, A_W2:A_W2 + E]
            q_v = cA[0:Wn + 1, A_Q:A_Q + 1]

            def xb_v(b):
                if b == 0:
                    return cA[0:Wn + 1, A_XB0:A_XB0 + K]
                return cB[0:Wn + 1, B_XB1:B_XB1 + K]

            def ss_v(j):  # [128,128] weights, sgn at col j
                return cA[:, A_SS + 128 - j:A_SS + 256 - j]

            def xto_v(b, h):
                c = B_XTO + (2 * b + h) * (Wn + 1)
                return cB[:, c:c + Wn + 1]

            def bkt_v(jh):
                return cB[:, B_BKT + jh * K:B_BKT + (jh + 1) * K]

            ident_v = cB[:, B_ID:B_ID + E]

            def fcw_v(ih):
                return cB[:, B_FCW + ih * O:B_FCW + (ih + 1) * O]

            fcb_v = cB[0:O, B_FCB:B_FCB + 1]

            # per-batch state
            lt_b = [None] * BPC
            rtb_f = [None] * BPC
            srb_f = [None] * BPC
            pT = [[None, None] for _ in range(BPC)]
            tt = [[None, None] for _ in range(BPC)]
            ps_so_t = [None] * BPC

            ot2 = consts.tile([O, BPC * Wn], FP32, tag="ot2")

            def emit_proj(b):
                ps_lr = psA.tile([E, 2 * K], FP32, tag="ps_lr")
                nc.tensor.matmul(ps_lr[:, 0:K], w1at_v, xb_v(b)[0:Wn, :],
                                 start=True, stop=True)
                nc.tensor.matmul(ps_lr[:, K:2 * K], w2bt_v, xb_v(b),
                                 start=True, stop=True)
                lt_b[b] = setup.tile([E, K], BF16, tag="lt_b", name=f"lt_b{b}")
                nc.scalar.copy(lt_b[b][:], ps_lr[:, 0:K])
                rtb_f[b] = setup.tile([E, K], FP32, tag="rtb_f", name=f"rtb_f{b}")
                nc.vector.tensor_copy(rtb_f[b][:], ps_lr[:, K:2 * K])
                # srb columns: srb[j] = sum_w q[w] xb[w, j]
                ps_so_t[b] = psM.tile([128, 66], FP32, tag="ps_so",
                                      name=f"ps_so{b}")
                for jh in range(2):
                    nc.tensor.matmul(ps_so_t[b][:, 64 + jh:65 + jh],
                                     xb_v(b)[:, 128 * jh:128 * jh + 128],
                                     q_v, start=True, stop=True)
                srb_f[b] = setup.tile([K // 2, 2], FP32, tag="srb_f", name=f"srb_f{b}")
                nc.vector.tensor_copy(srb_f[b][:], ps_so_t[b][:, 64:66])

            def emit_gen(b, jh):
                ps_sc = psS.tile([K // 2, K], FP32, tag="ps_sc")
                for j in range(128):
                    J = 128 * jh + j
                    idx = j % 16
                    tj = trelu.tile([E, K], BF16, tag="tj")
                    if idx in ACT_SET:
                        nc.scalar.activation(
                            tj[:], lt_b[b][:], AF.Relu,
                            bias=rtb_f[b][:, J:J + 1], scale=1.0)
                    elif idx in POOL_SET:
                        nc.gpsimd.tensor_scalar(
                            out=tj[:], in0=lt_b[b][:],
                            scalar1=rtb_f[b][:, J:J + 1], scalar2=0.0,
                            op0=ALU.add, op1=ALU.max)
                    else:
                        nc.vector.tensor_scalar(
                            out=tj[:], in0=lt_b[b][:],
                            scalar1=rtb_f[b][:, J:J + 1], scalar2=0.0,
                            op0=ALU.add, op1=ALU.max)
                    nc.tensor.matmul(ps_sc[:], ss_v(j), tj[:],
                                     start=(j == 0), stop=False,
                                     skip_group_check=True)
                # close the chain by accumulating bias_kk^T via identity
                nc.tensor.matmul(ps_sc[:], ident_v, bkt_v(jh),
                                 start=False, stop=True,
                                 skip_group_check=True)
                pT[b][jh] = etiles.tile([K // 2, K], BF16, tag="pT", name=f"pT{b}_{jh}")
                nc.scalar.activation(pT[b][jh][:], ps_sc[:], AF.Exp,
                                     bias=srb_f[b][:, jh:jh + 1], scale=1.0)

            def emit_attn(b):
                ps_n2 = psM.tile([K // 2, 2 * (Wn + 1)], FP32, tag="ps_n2",
                                 name=f"ps_n2{b}")
                for ih in range(2):
                    ps_num = ps_n2[:, (Wn + 1) * ih:(Wn + 1) * (ih + 1)]
                    for jh in range(2):
                        nc.tensor.matmul(
                            ps_num, pT[b][jh][:, 128 * ih:128 * ih + 128],
                            xto_v(b, jh), start=(jh == 0), stop=(jh == 1))
                    rcol = small.tile([K // 2, 1], FP32, tag=f"rcol{ih}",
                                      name=f"rcol{b}_{ih}")
                    nc.vector.reciprocal(rcol[:], ps_num[:, Wn:Wn + 1])
                    tt[b][ih] = small.tile([K // 2, Wn], BF16, tag=f"tt{ih}", name=f"tt{b}_{ih}")
                    nc.scalar.activation(tt[b][ih][:], ps_num[:, 0:Wn],
                                         AF.Tanh, scale=rcol[:])

            def emit_fc(b):
                ps_o = ps_so_t[b][0:O, 0:Wn]
                for ih in range(2):
                    nc.tensor.matmul(ps_o, fcw_v(ih), tt[b][ih][:],
                                     start=(ih == 0), stop=(ih == 1))
                nc.scalar.activation(ot2[:, Wn * b:Wn * (b + 1)], ps_o,
                                     AF.Identity, bias=fcb_v)

            # software-pipelined emission
            emit_proj(0)
            emit_gen(0, 0)
            emit_gen(0, 1)
            emit_proj(1)
            emit_gen(1, 0)
            emit_attn(0)
            emit_gen(1, 1)
            emit_fc(0)
            emit_attn(1)
            emit_fc(1)
            nc.gpsimd.dma_start(out=d_out.ap(), in_=ot2[:])

    nc.compile()
    return nc


_NC_CACHE = {}


def _get_program():
    if "nc" not in _NC_CACHE:
        _NC_CACHE["nc"] = _build_program()
    return _NC_CACHE["nc"]


def _host_prep(x, lin_w, lin_b, a, bias_kk, fc_w, fc_b):
    f32 = np.float32
    bf16 = ml_dtypes.bfloat16
    x = np.ascontiguousarray(x, f32)
    aa = (np.abs(a) * (1.0 - ALPHA)).astype(f32)
    sgn = np.sign(a).astype(f32)
    alpha_p = ALPHA / (1.0 - ALPHA)

    w1at = (lin_w[:, :Wn] * aa[:, None]).T.astype(f32)          # [64,128]
    w2t = (lin_w[:, Wn:] * aa[:, None]).T                        # [64,128]
    bt = (lin_b * aa)[None, :]
    w2bt = np.concatenate([w2t, bt], 0).astype(f32)              # [65,128]
    q = (alpha_p * (w2bt @ sgn)).astype(f32)                     # [65]
    bkkt = bias_kk.T.astype(f32)                                 # [256,256]
    fcw2t = (0.5 * fc_w).T.astype(f32)                           # [256,64]
    fcb2 = (fc_b + 0.5 * fc_w.sum(1)).astype(f32)                # [64]

    packA = np.zeros((128, A_COLS), f32)
    packA[0:Wn, A_W1:A_W1 + E] = w1at
    packA[0:Wn + 1, A_W2:A_W2 + E] = w2bt
    packA[0:Wn + 1, A_Q] = q
    packA[:, A_SS + 128] = sgn

    packB_shared = np.zeros((128, B_COLS), f32)
    packB_shared[:, B_BKT:B_BKT + K] = bkkt[0:128, :]
    packB_shared[:, B_BKT + K:B_BKT + 2 * K] = bkkt[128:256, :]
    packB_shared[:, B_ID:B_ID + E] = np.eye(128, dtype=f32)
    packB_shared[:, B_FCW:B_FCW + O] = fcw2t[0:128, :]
    packB_shared[:, B_FCW + O:B_FCW + 2 * O] = fcw2t[128:256, :]
    packB_shared[0:O, B_FCB] = fcb2

    in_maps = []
    for c in range(N_CORES):
        pa = packA.copy()
        pb = packB_shared.copy()
        for i in range(BPC):
            xb = x[BPC * c + i]                                  # [64,256]
            xb1 = np.concatenate([xb, np.ones((1, K), f32)], 0)  # [65,256]
            vt = xb.T                                            # [256,64]
            xto2 = np.concatenate([vt, np.full((K, 1), 2.0, f32)], 1)
            if i == 0:
                pa[0:Wn + 1, A_XB0:A_XB0 + K] = xb1
            else:
                pb[0:Wn + 1, B_XB1:B_XB1 + K] = xb1
            for h in range(2):
                c0 = B_XTO + (2 * i + h) * (Wn + 1)
                pb[:, c0:c0 + Wn + 1] = xto2[128 * h:128 * h + 128, :]
        in_maps.append({
            "packA": np.ascontiguousarray(pa.astype(bf16)),
            "packB": np.ascontiguousarray(pb.astype(bf16)),
        })
    return in_maps


def kernel(x, lin_w, lin_b, a, bias_kk, fc_w, fc_b, _trace=False):
    nc = _get_program()
    in_maps = _host_prep(np.asarray(x), np.asarray(lin_w), np.asarray(lin_b),
                         np.asarray(a), np.asarray(bias_kk),
                         np.asarray(fc_w), np.asarray(fc_b))
    res = run_bass_kernel_spmd(nc, in_maps, list(range(N_CORES)),
                               trace=_trace)
    out = np.empty((B, Wn, O), np.float32)
    for c in range(N_CORES):
        o = res.results[c]["outp"]          # (O, BPC*Wn)
        for i in range(BPC):
            out[BPC * c + i] = o[:, Wn * i:Wn * (i + 1)].T
    if _trace:
        return out, res
    return out
